# revision 37
# baseline (speedup 1.0000x reference)
"""Trainium2 Bass kernel for nn_ComplexPtreeLayer (3-level tree message passing).

Math: for the structured inputs produced by the problem's setup_inputs()
(order matrices are cyclic within-group permutations, seg = i//4, B == K == 4),
each tree layer collapses exactly:

    out_g = (sum of the 4 rows in group g) @ (Wzf @ sum_k Wz_k)^T + c
    c     = 4 * (sum_k bz_k @ Wzf^T + bzf)

because summing a group's 4 rows makes every cyclic slot-gather contribute the
same group sum. Chaining 3 levels with Mc = Wzf @ sum_k Wz_k, the per-level
matmuls (left-multiplications) commute with the pools (right-multiplications
by 0/1 block matrices), so the whole network collapses further to a single
matmul against the 64-leaf tree sums:

    out^T = Mc^3 @ pool64(x^T) + c_final * 1^T
    c_final = c + 4*Mc@c + 16*Mc@Mc@c

The kernel verifies the structural assumptions on the actual inputs at run
time and falls back to an exact dense numpy evaluation of the reference
semantics if they do not hold.

Sharding: data-parallel over trees. 65536 leaves / 8 cores = 8192 consecutive
leaves (= 128 whole trees) per core; weights replicated; no collectives.

Precision: x streams to the device in fp8-e4m3 quantized with error feedback
along each tree's 64 pooled rows (the rounding residual of row i is carried
into row i+1 before quantizing), so the device-computed 64-row sums keep
close to fp16 accuracy while HBM traffic halves vs fp16 (the kernel is
DMA-bound; measured end-to-end rel err ~2.8e-3 vs the 2e-2 gate). Mc^3 stays
fp16.

Device layout is "transposed" (hidden dim on partitions): stage 1 is a PE
matmul with x tiles as the stationary operand and a [128,2] per-tree fp8
pooling matrix as the moving operand (sums each tree's 64 leaves and
transposes in one pass, fp32 PSUM accumulate); stage 2 multiplies by Mc^3 in
fp16. The PE instruction stream is software-pipelined one chunk ahead
(stage 1 of chunk s+1 is emitted before stage 2 of chunk s) so the final
chunk's pooling dispatches the instant its DMA semaphore fires. Outputs are
staged tree-major into two SBUF accumulators: z_main (chunks 0..3, one ACT
DMA that fires mid-stream) and z_tail (chunks 4..6), which splits into an
early ACT DMA (chunks 4-5) and a tiny final SP DMA gated only on the last
3-tile chunk's DVE copy -- the only work exposed after the final x byte is
that chunk's stage1 -> s64 copy -> stage2 -> z copy -> small DMA chain. The
c_final bias is added on the host.
"""

import sys

import numpy as np

for _p in ("/opt/trn_rl_repo",):
    if _p not in sys.path:
        sys.path.append(_p)

H = 512
N0 = 65536
NCORES = 8
ROWS = N0 // NCORES          # 8192 rows per core
G3 = ROWS // 64              # 128 output rows (trees) per core
B = 4
K = 4
# chunk sizes in 128-row x-tiles. Every chunk's transfer stays >= the ~625ns
# HWDGE pacing of its successor (gap-free DMA bus); the last chunk is small
# so the post-stream dependency chain is short.
CHUNK_TILES = [11, 11, 12, 14, 8, 5, 3]
NMAIN = 4                     # leading chunks staged into z_main
NDUMMY = 5                    # fence-lane rotation dummies (see below)
GMAIN = 2 * sum(CHUNK_TILES[:NMAIN])
GTAIL = G3 - GMAIN
GLAST = 2 * CHUNK_TILES[-1]   # trees in the final, latency-critical chunk
assert sum(CHUNK_TILES) == ROWS // 128

_RUNNER = None


def _check_structured(x, Wz, bz, Wzf, bzf, node_idx, order1, order2, order3,
                      seg1, seg2, seg3):
    if node_idx.shape != (N0,) or x.shape != (N0, H):
        return False
    if not np.array_equal(node_idx, np.arange(N0, dtype=node_idx.dtype)):
        return False
    for o, s, n in ((order1, seg1, N0), (order2, seg2, N0 // B),
                    (order3, seg3, N0 // B // B)):
        if o.shape != (K, n) or s.shape != (n,):
            return False
        i = np.arange(n)
        m = np.arange(K)[:, None]
        exp = (i // B) * B + (i[None, :] % B + m) % B + 1
        if not np.array_equal(o, exp.astype(o.dtype)):
            return False
        if not np.array_equal(s, (i // B).astype(s.dtype)):
            return False
    return True


def _fallback(x, Wz, bz, Wzf, bzf, node_idx, order1, order2, order3,
              seg1, seg2, seg3):
    """Exact dense evaluation of the reference semantics (numpy, fp32)."""
    data = x[node_idx]
    for order, seg in ((order1, seg1), (order2, seg2), (order3, seg3)):
        n = order.shape[1]
        padded = np.concatenate([np.zeros((1, H), data.dtype), data], axis=0)
        acc = np.zeros((n, H), np.float32)
        for k in range(K):
            contrib = padded[order[k]] @ Wz[k].T + bz[k]
            contrib[order[k] == 0] = 0.0
            acc += contrib
        z = acc @ Wzf.T + bzf
        out = np.zeros((n // B, H), np.float32)
        np.add.at(out, seg, z)
        data = out
    return data


def _quant_fp8_errfb(x):
    """fp8-e4m3 quantization with error feedback along each tree's 64 rows.

    Carrying the rounding residual row-to-row makes each 64-row pooled sum
    accurate to ~one final-row rounding instead of sqrt(64) accumulated
    roundings; the device still reads and reduces every row.
    """
    import ml_dtypes

    xg = x.reshape(-1, 64, H)
    q = np.empty(xg.shape, ml_dtypes.float8_e4m3)
    carry = np.zeros((xg.shape[0], H), np.float32)
    for i in range(64):
        v = xg[:, i, :] + carry
        qv = v.astype(ml_dtypes.float8_e4m3)
        carry = v - qv.astype(np.float32)
        q[:, i, :] = qv
    return np.ascontiguousarray(q.reshape(-1, H))


def _build_runner():
    import concourse.bacc as bacc
    import concourse.bass as bass
    import concourse.mybir as mybir
    import concourse.tile as tile

    f8 = mybir.dt.float8e4
    f16 = mybir.dt.float16
    f32 = mybir.dt.float32

    nc = bacc.Bacc("TRN2", target_bir_lowering=False, debug=False,
                   num_devices=NCORES)

    xs = nc.dram_tensor("xs", [ROWS, H], f8, kind="ExternalInput")
    mc3t = nc.dram_tensor("mc3t", [H, H], f16, kind="ExternalInput")
    # outputs are tree-major: out[p, g, jo] = out^T[jo*128 + p, tree g]
    out_m = nc.dram_tensor("out_m", [128, GMAIN, 4], f16, kind="ExternalOutput")
    out_t = nc.dram_tensor("out_t", [128, GTAIL, 4], f16, kind="ExternalOutput")

    # rows = 128*(tile index) + p; chunks are runs of consecutive tiles
    xs_v = xs.ap().rearrange("(t p) h -> t p h", p=128)
    mc3t_v = mc3t.ap().rearrange("(i p) h -> p i h", p=128)
    nchunks = len(CHUNK_TILES)
    tcmax = max(CHUNK_TILES) * 2

    with tile.TileContext(nc) as tc:
        with (
            tc.tile_pool(name="consts", bufs=1) as consts,
            tc.tile_pool(name="xpool", bufs=4) as xpool,
            tc.tile_pool(name="s64p", bufs=2) as s64p,
            tc.tile_pool(name="zp", bufs=1) as zp,
            tc.tile_pool(name="psum1", bufs=2, space=bass.MemorySpace.PSUM) as psum1,
            tc.tile_pool(name="psum2", bufs=2, space=bass.MemorySpace.PSUM) as psum2,
        ):
            # first big x load ahead of everything so the DMA pipe fills.
            # SP carries the x loads (plus the two tail out DMAs at the end).
            xt0 = xpool.tile([128, CHUNK_TILES[0], H], f8, tag="xt", name="xt0")
            nc.sync.dma_start(
                xt0[:], xs_v[0:CHUNK_TILES[0]].rearrange("t p h -> p t h"))

            # consts ride the ACT queue; mc3t in one DMA
            mc3t_sb = consts.tile([128, 4, H], f16, tag="mc3t", name="mc3t_sb")
            nc.scalar.dma_start(mc3t_sb[:], mc3t_v)
            # the 0/1 pooling matrix is built on-device (Pool engine is
            # otherwise idle), keeping its bytes off the serial DMA bus
            p4_sb = consts.tile([128, 2], f8, tag="p4", name="p4_sb")
            nc.gpsimd.memset(p4_sb[:], 0.0)
            nc.gpsimd.memset(p4_sb[0:64, 0:1], 1.0)
            nc.gpsimd.memset(p4_sb[64:128, 1:2], 1.0)

            # tree-major z accumulators ([128, tree, jo]): tree ranges are
            # contiguous so the tail output can split into an early DMA
            # (chunks 4-5) and a tiny final DMA (last chunk only).
            z_main = zp.tile([128, GMAIN, 4], f16, tag="zm", name="z_main")
            z_tail = zp.tile([128, GTAIL, 4], f16, tag="zt", name="z_tail")

            # x chunk loads (all on SP, in order)
            xts = [xt0]
            t0 = CHUNK_TILES[0]
            for s, ctiles in enumerate(CHUNK_TILES[1:], start=1):
                xt = xpool.tile([128, ctiles, H], f8,
                                tag="xt" if ctiles >= 5 else f"xs{s}",
                                name=f"xt{s}")
                nc.sync.dma_start(
                    xt[:], xs_v[t0:t0 + ctiles].rearrange("t p h -> p t h"))
                xts.append(xt)
                t0 += ctiles

            def stage1(s):
                # matmul: ps[h, tree] = sum_p x[p, h] * P4[p, tree]
                ctiles = CHUNK_TILES[s]
                ps = psum1.tile([128, 4, tcmax], f32, tag="ps", name=f"ps{s}")
                for t in range(ctiles):
                    for j in range(4):
                        nc.tensor.matmul(
                            ps[:, j, 2 * t:2 * t + 2],
                            xts[s][:, t, j * 128:(j + 1) * 128],
                            p4_sb[:],
                            start=True, stop=True,
                        )
                return ps

            gm = 0
            gt = 0
            # PE order software-pipelines one chunk ahead: stage 1 of chunk
            # s+1 is emitted before stage 2 of chunk s, so when the last x
            # chunk's DMA sem fires its stage-1 matmuls dispatch immediately
            # instead of queueing behind the previous chunk's stage 2.
            pss = [stage1(0)]
            for s, ctiles in enumerate(CHUNK_TILES):
                tcs = 2 * ctiles                     # trees in this chunk
                if s + 1 < nchunks:
                    pss.append(stage1(s + 1))

                s64 = s64p.tile([128, 4, tcmax], f16, tag="s64", name=f"s64_{s}")
                nc.vector.tensor_copy(s64[:, :, :tcs], pss[s][:, :, :tcs])

                # ---- stage 2: out^T[h_out, tree] = Mc^3 @ s64 ----
                ps2 = psum2.tile([128, 4, tcmax], f32, tag="ps2", name=f"ps2_{s}")
                for jo in range(4):
                    for i in range(4):
                        nc.tensor.matmul(
                            ps2[:, jo, :tcs],
                            mc3t_sb[:, i, jo * 128:(jo + 1) * 128],
                            s64[:, i, :tcs],
                            start=(i == 0), stop=(i == 3),
                        )

                # ---- z staging (tree-major; copy APs transpose jo/tree) ----
                # Main chunks and tail chunks 4-5 stage on ACT; the final
                # chunk stages on the otherwise-idle DVE so the ACT queue
                # cannot delay the critical chain.
                src = ps2[:, :, :tcs].rearrange("p a b -> p b a")
                if s < NMAIN:
                    nc.scalar.add(z_main[:, gm:gm + tcs], src, 0.0)
                    gm += tcs
                    if s == NMAIN - 1:
                        nc.scalar.dma_start(out_m.ap(), z_main[:])
                elif s < nchunks - 1:
                    nc.scalar.add(z_tail[:, gt:gt + tcs], src, 0.0)
                    gt += tcs
                    if s == nchunks - 2:
                        # early tail output (chunks 4-5) rides ACT behind its
                        # own adds, leaving SP free for the final DMA
                        gsplit = GTAIL - GLAST
                        nc.scalar.dma_start(out_t.ap()[:, :gsplit],
                                            z_tail[:, :gsplit])
                else:
                    nc.vector.tensor_copy(z_tail[:, gt:gt + tcs], src)
                    gt += tcs

            # Three 1-descriptor dummy loads rotate Tile's round-robin
            # HWDGE lane assignment so the final out_t DMA lands on the lane
            # whose completion fence the epilogue checks near-last -- the
            # other lane fences then retire while waiting for it instead of
            # after it (-147ns, measured). Emitted here so their transfers
            # ride the idle post-stream bus; independent scratch tiles keep
            # them off every dependency chain.
            # SP placement keeps the dummies' HWDGE slots behind every x
            # chunk's, so the early stream is unperturbed (ACT placement
            # costs ~+50ns of early HWDGE contention).
            for _d in range(NDUMMY):
                _scr = consts.tile([1, 2], f8, tag=f"scr{_d}", name=f"scr{_d}")
                nc.sync.dma_start(_scr[:], xs.ap()[0:1, 0:2])

            # tiny final DMA on SP (idle after the x loads), gated only on
            # the last chunk's DVE copy
            gsplit = GTAIL - GLAST
            nc.sync.dma_start(out_t.ap()[:, gsplit:], z_tail[:, gsplit:])

    nc.compile()
    return nc


def kernel(x, Wz, bz, Wzf, bzf, node_idx, order1, order2, order3,
           seg1, seg2, seg3):
    x = np.asarray(x, dtype=np.float32)
    Wz = np.asarray(Wz, dtype=np.float32)
    bz = np.asarray(bz, dtype=np.float32)
    Wzf = np.asarray(Wzf, dtype=np.float32)
    bzf = np.asarray(bzf, dtype=np.float32)
    node_idx = np.asarray(node_idx)
    orders = [np.asarray(o) for o in (order1, order2, order3)]
    segs = [np.asarray(s) for s in (seg1, seg2, seg3)]

    if not _check_structured(x, Wz, bz, Wzf, bzf, node_idx, *orders, *segs):
        return _fallback(x, Wz, bz, Wzf, bzf, node_idx, *orders, *segs)

    # host-side weight folding (tiny)
    Wsum = Wz.sum(axis=0, dtype=np.float64)
    Mc = (Wzf.astype(np.float64) @ Wsum).astype(np.float32)
    c = 4.0 * (bz.sum(axis=0, dtype=np.float64) @ Wzf.astype(np.float64).T
               + bzf.astype(np.float64))
    Mc64 = Mc.astype(np.float64)
    cfinal = (c + 4.0 * (Mc64 @ (c + 4.0 * (Mc64 @ c)))).astype(np.float32)

    Mc3 = (Mc64 @ Mc64 @ Mc64).astype(np.float32)
    x8 = _quant_fp8_errfb(x)
    mc3t16 = np.ascontiguousarray(Mc3.T.astype(np.float16))   # [h_in, h_out]
    global _RUNNER
    if _RUNNER is None:
        _RUNNER = _build_runner()
    nc = _RUNNER

    try:
        named = _run_fast(nc, x8, mc3t16)
    except Exception:
        from concourse.bass_utils import run_bass_kernel_spmd

        in_maps = [
            {"xs": x8[i * ROWS:(i + 1) * ROWS], "mc3t": mc3t16}
            for i in range(NCORES)
        ]
        res = run_bass_kernel_spmd(nc, in_maps, core_ids=list(range(NCORES)))
        named = {
            n: np.stack([r[n] for r in res.results], axis=0)
            for n in ("out_m", "out_t")
        }

    # out_*[core][p, g, jo] holds out^T[jo*128 + p, g] in fp16 (g = tree
    # local to the core; main trees then tail trees).
    out_g = np.concatenate(
        [np.asarray(named["out_m"]).reshape(NCORES, 128, GMAIN, 4),
         np.asarray(named["out_t"]).reshape(NCORES, 128, GTAIL, 4)],
        axis=2)                                            # [8, 128, G3, 4]
    # [core, p, g, jo] -> [core, g, jo, p] -> [core*g, jo*128+p]
    out = (out_g.astype(np.float32).transpose(0, 2, 3, 1)
           .reshape(NCORES * G3, H))
    out += cfinal[None, :]
    return out


_SHARDED = None


def _run_fast(nc, x8, mc3t16):
    """Execute via a cached shard_map'd PJRT callable (one trace/compile,
    reused across calls). Mirrors bass2jax.run_bass_via_pjrt's SPMD path."""
    global _SHARDED
    import jax
    from jax.sharding import Mesh, PartitionSpec
    from concourse import mybir
    from concourse.bass2jax import (_bass_exec_p, install_neuronx_cc_hook,
                                    partition_id_tensor)

    if _SHARDED is None:
        install_neuronx_cc_hook()
        pname = nc.partition_id_tensor.name if nc.partition_id_tensor else None
        in_names, out_names, out_avals = [], [], []
        for alloc in nc.m.functions[0].allocations:
            if not isinstance(alloc, mybir.MemoryLocationSet):
                continue
            name = alloc.memorylocations[0].name
            if alloc.kind == "ExternalInput":
                if name != pname:
                    in_names.append(name)
            elif alloc.kind == "ExternalOutput":
                out_names.append(name)
                out_avals.append(jax.core.ShapedArray(
                    tuple(alloc.tensor_shape), mybir.dt.np(alloc.dtype)))
        n_params = len(in_names)
        in_names_all = list(in_names) + list(out_names)
        if pname is not None:
            in_names_all.append(pname)

        def _body(*args):
            operands = list(args)
            if pname is not None:
                operands.append(partition_id_tensor())
            return tuple(_bass_exec_p.bind(
                *operands,
                out_avals=tuple(out_avals),
                in_names=tuple(in_names_all),
                out_names=tuple(out_names),
                lowering_input_output_aliases=(),
                sim_require_finite=True,
                sim_require_nnan=True,
                nc=nc,
            ))

        devices = jax.devices()[:NCORES]
        mesh = Mesh(np.asarray(devices), ("core",))
        specs = (PartitionSpec("core"),)
        kw = dict(mesh=mesh,
                  in_specs=specs * (n_params + len(out_names)),
                  out_specs=specs * len(out_names))
        try:
            smap = jax.shard_map(_body, check_vma=False, **kw)
        except TypeError:
            smap = jax.shard_map(_body, check_rep=False, **kw)
        sharded = jax.jit(smap, keep_unused=True)
        _SHARDED = (sharded, in_names, out_names, out_avals)

    sharded, in_names, out_names, out_avals = _SHARDED
    per_core = {
        "xs": x8,                                      # concat of shards == x
        "mc3t": np.concatenate([mc3t16] * NCORES, axis=0),
    }
    ins = [per_core[n] for n in in_names]
    zeros = [np.zeros((NCORES * a.shape[0], *a.shape[1:]), a.dtype)
             for a in out_avals]
    out_arrs = sharded(*ins, *zeros)
    return dict(zip(out_names, out_arrs))
